# revision 5
# baseline (speedup 1.0000x reference)
"""nn_BitConv2d Trainium2 kernel v3 — 8-core data-parallel over batch.

Math: y = 16 * sum_k 2^(7-k) * trunc(conv2d(bit_k(x)/16, W)) + bias.

Per core (2 of 16 images), per bit-plane conv:
- bits 0-3: fp16 matmuls. Plane bits are extracted by ONE int16 shl+and DVE
  op (2x mode) writing the fp16 exponent pattern 0x0400 => plane value
  2^-14; weights are fp16(W*2^10) so products land on the reference scale.
  eps = 2^-11.
- bits 4-7: fp8e4 DoubleRow matmuls (K=256 per instruction). Plane bytes
  0x08 (=2^-6) are built from packed uint8 x pairs by one int16 shl+and op;
  weights e4m3(4W). eps = 2^-4, acceptable for low-significance bits.
  Simulated rel err of this mix: 6.2e-3 (gate 2e-2).
- 3x3 conv = 9 shifted matmuls per (co_t, 8-row sp tile), PSUM pairs
  [128,2,512] + single, 7 banks/group, pool-rotated for overlap.
- trunc epilogue: sg=Sign(ps) on ACT; ONE DVE stt (-0.5*sg + ps) with int32
  output (hw f32->int cast is round-to-nearest => exact trunc); Horner
  y=2y+t on DVE.
- Bit-plane extraction runs one (img,bit) step AHEAD of its matmuls in DVE
  program order, so the PE never waits on extraction at bit boundaries.
"""
import sys

if "/opt/trn_rl_repo" not in sys.path:
    sys.path.insert(0, "/opt/trn_rl_repo")

import numpy as np
import ml_dtypes
from contextlib import ExitStack

import concourse.bacc as bacc
import concourse.tile as tile
from concourse import mybir
from concourse.bass_utils import run_bass_kernel_spmd

AL = mybir.AluOpType
AF = mybir.ActivationFunctionType
F32 = mybir.dt.float32
F16 = mybir.dt.float16
I16 = mybir.dt.int16
I32 = mybir.dt.int32
U8 = mybir.dt.uint8
FP8 = mybir.dt.float8e4
E4 = ml_dtypes.float8_e4m3
DR = mybir.MatmulPerfMode.DoubleRow

N_CORES = 8
B = 16
B_PER_CORE = B // N_CORES
CIN = 256
COUT = 256
H = W = 56
HW = H * W
NBITS = 8
NROW = 8
NSP = H // NROW          # 7
NFREE = NROW * W         # 448
PADH = 58
PADW = 60                # interior at cols 2..57; cols 1,58 zero
M_FP16 = 4               # bits 0..3 fp16, bits 4..7 fp8-DR


def _build(reps=None):
    nc = bacc.Bacc("TRN2", target_bir_lowering=False, debug=False)

    x_d = nc.dram_tensor("x", [B_PER_CORE, CIN, HW], F32, kind="ExternalInput")
    w16_d = nc.dram_tensor("w16", [2 * 9 * 2, 128, 128], F16, kind="ExternalInput")
    w8_d = nc.dram_tensor("w8", [2 * 9 * 2, 128, 128], FP8, kind="ExternalInput")
    w8q_d = nc.dram_tensor("w8q", [2 * 9 * 2, 128, 128], FP8, kind="ExternalInput")
    b_d = nc.dram_tensor("bias", [COUT], F32, kind="ExternalInput")
    y_d = nc.dram_tensor("y", [B_PER_CORE, COUT, HW], F32, kind="ExternalOutput")

    with tile.TileContext(nc) as tc, ExitStack() as ctx:
        const = ctx.enter_context(tc.tile_pool(name="const", bufs=1))
        pairpool = ctx.enter_context(tc.tile_pool(name="pspair", bufs=3,
                                                  space="PSUM"))
        singpool = ctx.enter_context(tc.tile_pool(name="pssing", bufs=2,
                                                  space="PSUM"))
        tmppool = ctx.enter_context(tc.tile_pool(name="tmp", bufs=4))

        # weights: [ci, co_t, tap, ci_t, co]
        w16_sb = const.tile([128, 2, 9, 2, 128], F16, name="w16_sb")
        nc.sync.dma_start(
            w16_sb[:].rearrange("k c n i m -> k (c n i) m"),
            w16_d.ap().rearrange("o k m -> k o m"))
        w8_sb = const.tile([128, 2, 9, 2, 128], FP8, name="w8_sb")
        nc.sync.dma_start(
            w8_sb[:].rearrange("k c n i m -> k (c n i) m"),
            w8_d.ap().rearrange("o k m -> k o m"))
        w8q_sb = const.tile([128, 2, 9, 2, 128], FP8, name="w8q_sb")
        nc.sync.dma_start(
            w8q_sb[:].rearrange("k c n i m -> k (c n i) m"),
            w8q_d.ap().rearrange("o k m -> k o m"))
        bias_sb = const.tile([128, 2], F32, name="bias_sb")
        nc.sync.dma_start(bias_sb[:], b_d.ap().rearrange("(c p) -> p c", p=128))

        # x as int16 (for fp16 planes) and packed u8 pairs (for fp8 planes)
        xi = const.tile([128, B_PER_CORE, 2, HW], I16, name="xi")
        xp = const.tile([128, B_PER_CORE, 2, HW], U8, name="xp")
        for img in range(B_PER_CORE):
            for ci_t in range(2):
                src = x_d.ap()[img, ci_t * 128:(ci_t + 1) * 128, :]
                nc.gpsimd.dma_start(xi[:, img, ci_t, :], src)
                nc.gpsimd.dma_start(xp[:, img, ci_t, :], src)

        y_acc = const.tile([128, B_PER_CORE, 2, HW], F32, name="y_acc")

        pl16 = [const.tile([128, 2, PADH, PADW], F16, name=f"pl16_{i}")
                for i in range(2)]
        pl8 = [const.tile([128, 2, PADH, PADW], FP8, name=f"pl8_{i}")
               for i in range(2)]
        for t in pl16 + pl8:
            nc.vector.memset(t[:], 0.0)

        # bits 0..4 are separate trunc'd convs; bits 5,6,7 are merged into
        # one untruncated conv of q = 4*bit5 + 2*bit6 + bit7 (fp8 subnormal
        # plane {0..7}*2^-9 -- PE-verified exact; weights e4m3(32W)).
        BITS_SEQ = [0, 1, 2, 3, 4, 567]
        steps = [(img, bit) for img in range(B_PER_CORE) for bit in BITS_SEQ]

        def extract(step_idx):
            img, bit = steps[step_idx]
            if bit < M_FP16:
                dst = pl16[step_idx % 2]
                interior = dst[:, :, 1:57, 2:58].bitcast(I16)
                # bit (7-bit) -> fp16 exponent pos 10: shl (bit+3), mask 0x0400
                nc.vector.tensor_scalar(interior, xi[:, img, :, :], bit + 3,
                                        0x0400, op0=AL.logical_shift_left,
                                        op1=AL.bitwise_and)
            else:
                dst = pl8[step_idx % 2]
                interior = dst[:, :, 1:57, 2:58].bitcast(I16)
                if bit == 567:  # q = x & 7 per byte -> subnormals {0..7}*2^-9
                    sh, mask = 0, 0x0707
                else:           # bit (7-bit) -> fp8 pos 3: shl (bit-4), 0x0808
                    sh, mask = bit - 4, 0x0808
                nc.vector.tensor_scalar(interior, xp[:, img, :, :].bitcast(I16),
                                        sh, mask,
                                        op0=AL.logical_shift_left,
                                        op1=AL.bitwise_and)
            return dst

        loop_ctx = tc.For_i(0, reps, 1) if reps else None
        if loop_ctx is not None:
            loop_ctx.__enter__()

        cur_plane = extract(0)
        for si, (img, bit) in enumerate(steps):
            plane = cur_plane
            if si + 1 < len(steps):
                cur_plane = extract(si + 1)    # lookahead: DVE does this early
            fp16 = bit < M_FP16
            merged = bit == 567

            for co_t in range(2):
                pairs = [pairpool.tile([128, 2, 512], F32, tag="pp",
                                       name=f"pp{si}_{co_t}_{j}")
                         for j in range(3)]
                sing = singpool.tile([128, 512], F32, tag="psg",
                                     name=f"psg{si}_{co_t}")

                def ps_ap(sp):
                    return (pairs[sp // 2][:, sp % 2, 0:NFREE]
                            if sp < 6 else sing[:, 0:NFREE])

                def epilogue(j):
                    # t = trunc(ps); y = 2y + t — issued as soon as the
                    # pair's accumulation completes (sp-major order) so the
                    # banks drain while later sp tiles are still computing.
                    # Merged 6+7 step skips trunc: y = 4y + ps.
                    if j < 3:
                        ps = pairs[j][:, :, 0:NFREE]
                        n = 2 * NFREE
                    else:
                        ps = sing[:, 0:NFREE]
                        n = NFREE
                    ysl = y_acc[:, img, co_t, j * 2 * NFREE: j * 2 * NFREE + n]
                    if merged:
                        nc.vector.scalar_tensor_tensor(
                            ysl, ysl, 8.0, ps, op0=AL.mult, op1=AL.add)
                        return
                    sg = tmppool.tile([128, n], F16, tag="sg",
                                      name=f"sg{si}_{co_t}_{j}")
                    nc.scalar.activation(sg[:], ps, AF.Sign)
                    t = tmppool.tile([128, n], I32, tag="t",
                                     name=f"t{si}_{co_t}_{j}")
                    nc.vector.scalar_tensor_tensor(
                        t[:], sg[:], -0.5, ps, op0=AL.mult, op1=AL.add)
                    if bit == 0:
                        nc.vector.tensor_scalar(ysl, t[:], 0.0, None, op0=AL.add)
                    else:
                        nc.vector.scalar_tensor_tensor(
                            ysl, ysl, 2.0, t[:], op0=AL.mult, op1=AL.add)

                # sp-major: each PSUM bank completes early; weight reloads
                # per matmul are hidden by the PE (measured: 502 vs 499 cyc)
                for sp in range(NSP):
                    tgt = ps_ap(sp)
                    wi = 0
                    for ky in range(3):
                        for kx in range(3):
                            if fp16:
                                for ci in range(2):
                                    lhsT = w16_sb[:, co_t, ky * 3 + kx, ci, :]
                                    rhs = plane[:, ci,
                                                sp * NROW + ky: sp * NROW + ky + NROW,
                                                kx + 1: kx + 1 + W]
                                    nc.tensor.matmul(
                                        tgt, lhsT, rhs,
                                        start=(wi == 0), stop=(wi == 17))
                                    wi += 1
                            else:
                                wsrc = w8q_sb if merged else w8_sb
                                lhsT = wsrc[:, co_t, ky * 3 + kx, :, :]
                                rhs = plane[:, :,
                                            sp * NROW + ky: sp * NROW + ky + NROW,
                                            kx + 1: kx + 1 + W]
                                nc.tensor.matmul(
                                    tgt, lhsT, rhs, perf_mode=DR,
                                    start=(wi == 0), stop=(wi == 8))
                                wi += 1
                    if sp % 2 == 1:
                        epilogue(sp // 2)
                    elif sp == 6:
                        epilogue(3)

            if bit == BITS_SEQ[-1]:
                for co_t in range(2):
                    ya = y_acc[:, img, co_t, :]
                    nc.scalar.activation(ya, ya, AF.Identity,
                                         bias=bias_sb[:, co_t:co_t + 1],
                                         scale=16.0)
                    nc.sync.dma_start(
                        y_d.ap()[img, co_t * 128:(co_t + 1) * 128, :], ya)

        if loop_ctx is not None:
            loop_ctx.__exit__(None, None, None)

    nc.compile()
    return nc


def _prep_weights(weight):
    """-> (w16 [2*9*2,128,128] f16 = fp16(W*2^10), w8 same layout e4m3(4W));
    lhsT layout [co_t, tap, ci_t, ci, co]."""
    w64 = weight.astype(np.float64)
    w16 = (w64 * 1024.0).astype(np.float16)
    w8 = (w64 * 4.0).astype(E4)
    w8q = (w64 * 32.0).astype(E4)

    def lay(p):
        v = p.reshape(2, 128, 2, 128, 9)           # co_t, co, ci_t, ci, k
        return np.ascontiguousarray(
            v.transpose(0, 4, 2, 3, 1).reshape(2 * 9 * 2, 128, 128))

    return lay(w16), lay(w8), lay(w8q)


_NC_CACHE = {}


def _get_nc():
    if "nc" not in _NC_CACHE:
        _NC_CACHE["nc"] = _build()
    return _NC_CACHE["nc"]


def kernel(x, weight, bias):
    x = np.ascontiguousarray(np.asarray(x, dtype=np.float32))
    weight = np.ascontiguousarray(np.asarray(weight, dtype=np.float32))
    bias = np.asarray(bias, dtype=np.float32)

    nc = _get_nc()
    w16, w8, w8q = _prep_weights(weight)
    bias_flat = np.ascontiguousarray(bias.reshape(COUT))

    in_maps = []
    for c in range(N_CORES):
        in_maps.append({
            "x": np.ascontiguousarray(
                x[c * B_PER_CORE:(c + 1) * B_PER_CORE].reshape(B_PER_CORE, CIN, HW)),
            "w16": w16,
            "w8": w8,
            "w8q": w8q,
            "bias": bias_flat,
        })

    def run_once():
        res = None
        for attempt in range(3):
            try:
                res = run_bass_kernel_spmd(nc, in_maps,
                                           core_ids=list(range(N_CORES)))
                break
            except Exception:
                if attempt == 2:
                    raise
                import time as _time
                _time.sleep(15.0 * (attempt + 1))
        assert res is not None
        return np.concatenate(
            [res.results[c]["y"].reshape(B_PER_CORE, COUT, H, W)
             for c in range(N_CORES)], axis=0)

    # first-execution-after-reset has (rarely) returned garbage from one
    # core; the kernel is deterministic, so run twice and compare.
    y = run_once()
    y2 = run_once()
    if not np.array_equal(y, y2):
        y3 = run_once()
        y = y3 if np.array_equal(y3, y) or np.array_equal(y3, y2) else y2
    return np.ascontiguousarray(y.astype(np.float32))
